# revision 21
# baseline (speedup 1.0000x reference)
"""PointSIFT res module on 8 trn2 cores.

Strategy
--------
Data-parallel over BT=16 point clouds: 2 clouds per core.

The three [1,2]-stride-2 convs in each stack have no nonlinearity between
them, so each stack collapses to y[n] = sum_s M_s @ feat[idx[n,s]] + c with
M_s = Wc_{s//4} @ Wb_{(s//2)%2} @ Wa_{s%2} (folded on host in float64).

Per cloud on device:
  1. O(N^2) octant neighbor search, bit-exact vs the jax reference:
     dist = (dx^2+dy^2)+dz^2 in fp32; octant bit b = (fp32(d+1) >= 1);
     invalid (dist<=1e-10 or dist>=jd) pushed to a clamped key region.
     Key G = oct<<28 + (bits(dist)-bits(1e-10)) is order-preserving in dist
     within each octant; per-octant argmin = uint32 wrap-subtract + min;
     indices recovered with one max_index pass; empty octants fall back to
     self (reference semantics).
  2. Transform-then-gather: w_s[j] = M_s @ feat[j] for all j via PE matmuls
     (tables in DRAM, row-major [9216, 128]: 8 slots + 1 self-correction
     slab c - Msum_xyz @ xyz[j]), one indirect-DMA row gather per stack,
     sum over 9 slots, then stack 2 likewise, ReLU, concat with relu(pts).
"""

import numpy as np

import concourse.bass as bass
import concourse.bacc as bacc
import concourse.mybir as mybir
import concourse.tile as tile
from concourse.bass_utils import run_bass_kernel_spmd
from concourse.masks import make_identity

P = 128
NT = 8
N = P * NT            # 1024 points per cloud
CLOUDS = 2            # clouds per core
NCORES = 8
EC = 64
OC = 128
NSLOT = 9             # 8 octant slots + 1 self-correction slab
ROWS = NSLOT * N      # rows in each w-table

F32 = mybir.dt.float32
I32 = mybir.dt.int32
U32 = mybir.dt.uint32

JD = float(np.float32(0.2 * 0.2))          # fp32 value of python 0.2*0.2
# fp32-exact (HW routes ALU immediates through fp32): bits(1e-10)+1.
# valid dist > 1e-10 => bits >= BASE_SUB, so db >= 0 always.
BASE_SUB = 786818816
CLAMP = 0.0625
FOUND_THR = 240000000                       # fp32-exact, between valid/invalid
AF = mybir.ActivationFunctionType
DEBUG_IDX = False
DEBUG_G = False
DEBUG_W = False


def _neighbor_search(nc, tc, sb, aug, bxr, byr, bzr, idxall, selfc, soffs,
                     obase, sentinel):
    """Per-cloud octant argmin. Writes idxall [128, NT, NSLOT] int32 with
    global row ids (slot*1024 + j)."""
    dbg_first = getattr(nc, "_dbg_first", [False])
    for nt in range(NT):
        xn = aug[:, nt, 1:2]
        yn = aug[:, nt, 2:3]
        zn = aug[:, nt, 3:4]

        dx = sb.tile([P, N], F32, tag="dx")
        dy = sb.tile([P, N], F32, tag="dy")
        dz = sb.tile([P, N], F32, tag="dz")
        nc.vector.tensor_scalar(out=dx[:], in0=bxr[:], scalar1=xn,
                                scalar2=None, op0=mybir.AluOpType.subtract)
        nc.vector.tensor_scalar(out=dy[:], in0=byr[:], scalar1=yn,
                                scalar2=None, op0=mybir.AluOpType.subtract)
        nc.vector.tensor_scalar(out=dz[:], in0=bzr[:], scalar1=zn,
                                scalar2=None, op0=mybir.AluOpType.subtract)

        sqx = sb.tile([P, N], F32, tag="sqx")
        sqy = sb.tile([P, N], F32, tag="sqy")
        sqz = sb.tile([P, N], F32, tag="sqz")
        # exact IEEE squares (ACT Square is table-approximated)
        nc.vector.tensor_tensor(out=sqx[:], in0=dx[:], in1=dx[:],
                                op=mybir.AluOpType.mult)
        nc.vector.tensor_tensor(out=sqy[:], in0=dy[:], in1=dy[:],
                                op=mybir.AluOpType.mult)
        nc.vector.tensor_tensor(out=sqz[:], in0=dz[:], in1=dz[:],
                                op=mybir.AluOpType.mult)

        # dist accumulates in-place over sqx, then markers+clamp in-place
        dist = sqx
        nc.vector.tensor_tensor(out=dist[:], in0=dist[:], in1=sqy[:],
                                op=mybir.AluOpType.add)
        nc.vector.tensor_tensor(out=dist[:], in0=dist[:], in1=sqz[:],
                                op=mybir.AluOpType.add)

        # invalid markers: dist <= 1e-10 (self/dup) or dist >= jd.
        # All DVE ALU arithmetic is fp32-domain, so the whole search runs on
        # genuine fp32 values: members of octant o get score dm + 0.0
        # (exact); everything else gets dm + 1 (>= 1 > any valid dist).
        i1 = sqy
        i2 = sqz
        nc.vector.tensor_scalar(out=i1[:], in0=dist[:], scalar1=1e-10,
                                scalar2=64.0, op0=mybir.AluOpType.is_le,
                                op1=mybir.AluOpType.mult)
        nc.vector.tensor_scalar(out=i2[:], in0=dist[:], scalar1=JD,
                                scalar2=64.0, op0=mybir.AluOpType.is_ge,
                                op1=mybir.AluOpType.mult)
        nc.vector.tensor_tensor(out=dist[:], in0=dist[:], in1=i1[:],
                                op=mybir.AluOpType.add)
        nc.vector.tensor_tensor(out=dist[:], in0=dist[:], in1=i2[:],
                                op=mybir.AluOpType.add)
        dm = dist

        # octant bits: b = (fp32(d + 1.0) >= 1.0)  (matches (d+1).int() trunc)
        bx, by, bz = dx, dy, dz
        nc.vector.tensor_scalar(out=bx[:], in0=dx[:], scalar1=1.0,
                                scalar2=1.0, op0=mybir.AluOpType.add,
                                op1=mybir.AluOpType.is_ge)
        nc.vector.tensor_scalar(out=by[:], in0=dy[:], scalar1=1.0,
                                scalar2=1.0, op0=mybir.AluOpType.add,
                                op1=mybir.AluOpType.is_ge)
        nc.vector.tensor_scalar(out=bz[:], in0=dz[:], scalar1=1.0,
                                scalar2=1.0, op0=mybir.AluOpType.add,
                                op1=mybir.AluOpType.is_ge)
        # oct = (2*bx + by)*2 + bz  (fp32 values 0..7)
        nc.vector.scalar_tensor_tensor(out=by[:], in0=bx[:], scalar=2.0,
                                       in1=by[:], op0=mybir.AluOpType.mult,
                                       op1=mybir.AluOpType.add)
        nc.vector.scalar_tensor_tensor(out=bz[:], in0=by[:], scalar=2.0,
                                       in1=bz[:], op0=mybir.AluOpType.mult,
                                       op1=mybir.AluOpType.add)
        octf = bz

        # per-octant: score = dm + [oct != o]; min; first-index via max_index
        m8 = sb.tile([P, 8], F32, tag="m8")
        idx8 = sb.tile([P, 8], U32, tag="idx8")
        for o in range(8):
            score = sb.tile([P, N], F32, tag="score")
            nc.vector.scalar_tensor_tensor(out=score[:], in0=octf[:],
                                           scalar=float(o), in1=dm[:],
                                           op0=mybir.AluOpType.not_equal,
                                           op1=mybir.AluOpType.add)
            nc.vector.tensor_reduce(out=m8[:, o:o + 1], in_=score[:],
                                    axis=mybir.AxisListType.X,
                                    op=mybir.AluOpType.min)
            mi8 = sb.tile([P, 8], U32, tag="mi8")
            nc.vector.max_index(out=mi8[:],
                                in_max=m8[:, o:o + 1].to_broadcast([P, 8]),
                                in_values=score[:])
            nc.vector.tensor_copy(out=idx8[:, o:o + 1], in_=mi8[:, 0:1])

        found = sb.tile([P, 8], I32, tag="found")
        nc.vector.tensor_scalar(out=found[:], in0=m8[:], scalar1=0.5,
                                scalar2=None, op0=mybir.AluOpType.is_lt)
        # self fallback + slot offsets; write into idxall[:, nt, :]
        dst = idxall[:, nt, 0:8]
        nc.vector.tensor_tensor(out=dst, in0=selfc[:, nt:nt + 1]
                                .to_broadcast([P, 8]).bitcast(I32),
                                in1=soffs[:, 0:8], op=mybir.AluOpType.add)
        idx8i = sb.tile([P, 8], I32, tag="idx8i")
        nc.vector.tensor_tensor(out=idx8i[:], in0=idx8[:].bitcast(I32),
                                in1=soffs[:, 0:8], op=mybir.AluOpType.add)
        nc.vector.copy_predicated(out=dst, mask=found[:], data=idx8i[:])
        nc.vector.tensor_scalar(out=idxall[:, nt, 8:9],
                                in0=selfc[:, nt:nt + 1], scalar1=8 * N,
                                scalar2=None, op0=mybir.AluOpType.add)


def _trace_core_program(nc):
    """Trace the full per-core tile program (2 clouds)."""
    xyz_d = nc.dram_tensor('xyz', [CLOUDS, N, 3], F32, kind="ExternalInput")
    pts_d = nc.dram_tensor('pts', [CLOUDS, N, EC], F32, kind="ExternalInput")
    m1t_d = nc.dram_tensor('m1t', [68, 8 * OC], F32, kind="ExternalInput")
    w01_d = nc.dram_tensor('w01', [4, OC], F32, kind="ExternalInput")
    b2t_d = nc.dram_tensor('b2t', [OC, 8 * OC], F32, kind="ExternalInput")
    a2t_d = nc.dram_tensor('a2t', [4, 8 * OC], F32, kind="ExternalInput")
    w02_d = nc.dram_tensor('w02', [4, OC], F32, kind="ExternalInput")
    out_d = nc.dram_tensor('out', [CLOUDS, N, OC + EC], F32,
                           kind="ExternalOutput")
    dbg_d = None
    if DEBUG_IDX:
        dbg_d = nc.dram_tensor('dbg_idx', [CLOUDS, P, NT, NSLOT], I32,
                               kind="ExternalOutput")
    dbgw_d = None
    if DEBUG_W:
        dbgw_d = nc.dram_tensor('dbg_w1', [ROWS, OC], F32,
                                kind="ExternalOutput")
        dbgn1_d = nc.dram_tensor('dbg_new1', [P, NT, OC], F32,
                                 kind="ExternalOutput")
        dbgg1_d = nc.dram_tensor('dbg_g1', [P, NT, NSLOT, OC], F32,
                                 kind="ExternalOutput")
        dbgw2_d = nc.dram_tensor('dbg_w2', [ROWS, OC], F32,
                                 kind="ExternalOutput")
        dbgg2_d = nc.dram_tensor('dbg_g2', [P, NT, NSLOT, OC], F32,
                                 kind="ExternalOutput")
        dbgn1t_d = nc.dram_tensor('dbg_n1t', [P, N], F32,
                                  kind="ExternalOutput")
    dbgg_d = None
    if DEBUG_G:
        dbgg_d = nc.dram_tensor('dbg_g', [P, N], I32, kind="ExternalOutput")
        dbgm_d = nc.dram_tensor('dbg_mins', [P, 8], I32,
                                kind="ExternalOutput")
        dbgt_d = nc.dram_tensor('dbg_tgt', [P, 8], I32, kind="ExternalOutput")
        dbgi_d = nc.dram_tensor('dbg_i8', [P, 8], I32, kind="ExternalOutput")
        nc._dbg_tensors = (dbgg_d, dbgm_d, dbgt_d, dbgi_d)
    nc._dbg_g = dbgg_d

    with tile.TileContext(nc) as tc:
        with tc.tile_pool(name="const", bufs=1) as cp, \
             tc.tile_pool(name="sb", bufs=2) as sb, \
             tc.tile_pool(name="big", bufs=1) as bigp, \
             tc.tile_pool(name="ps", bufs=2, space="PSUM") as ps, \
             tc.tile_pool(name="psmm", bufs=4, space="PSUM") as psmm, \
             tc.tile_pool(name="dram", bufs=1, space="DRAM") as dp:

            ident = cp.tile([P, P], F32)
            make_identity(nc, ident[:])
            m1t = cp.tile([68, 8 * OC], F32)
            nc.sync.dma_start(out=m1t[:], in_=m1t_d[:])
            w01 = cp.tile([4, OC], F32)
            nc.sync.dma_start(out=w01[:], in_=w01_d[:])
            b2t = cp.tile([OC, 8 * OC], F32)
            nc.sync.dma_start(out=b2t[:], in_=b2t_d[:])
            a2t = cp.tile([4, 8 * OC], F32)
            nc.sync.dma_start(out=a2t[:], in_=a2t_d[:])
            w02 = cp.tile([4, OC], F32)
            nc.sync.dma_start(out=w02[:], in_=w02_d[:])

            obase = cp.tile([P, 8], I32)
            nc.gpsimd.iota(out=obase[:], pattern=[[1, 8]], base=0,
                           channel_multiplier=0)
            nc.vector.tensor_scalar(out=obase[:], in0=obase[:], scalar1=28,
                                    scalar2=None,
                                    op0=mybir.AluOpType.logical_shift_left)
            soffs = cp.tile([P, 8], I32)
            nc.gpsimd.iota(out=soffs[:], pattern=[[N, 8]], base=0,
                           channel_multiplier=0)
            selfc = cp.tile([P, NT], I32)
            nc.gpsimd.iota(out=selfc[:], pattern=[[P, NT]], base=0,
                           channel_multiplier=1)
            sentinel = cp.tile([P, 8], U32)
            nc.gpsimd.memset(sentinel[:], 0xFFFFFFFF)

            for c in range(CLOUDS):
                # ---- load + build aug [p, nt, 68] = [1, xyz, pts] ----
                aug = sb.tile([P, NT, 68], F32, tag="aug")
                nc.gpsimd.memset(aug[:, :, 0:1], 1.0)
                nc.sync.dma_start(
                    out=aug[:, :, 1:4],
                    in_=xyz_d[c].rearrange("(nt p) c -> p nt c", p=P))
                nc.sync.dma_start(
                    out=aug[:, :, 4:68],
                    in_=pts_d[c].rearrange("(nt p) c -> p nt c", p=P))

                # ---- augT [68, 1024] via PE transposes ----
                augT = bigp.tile([68, N], F32, tag="augT")
                for nt in range(NT):
                    tp = ps.tile([P, P], F32, tag="tp")
                    nc.tensor.transpose(out=tp[:68, :], in_=aug[:, nt, :],
                                        identity=ident[:])
                    nc.scalar.copy(out=augT[:, nt * P:(nt + 1) * P],
                                   in_=tp[:68, :])

                # ---- broadcast coordinate rows ----
                bxr = bigp.tile([P, N], F32, tag="bxr")
                byr = bigp.tile([P, N], F32, tag="byr")
                bzr = bigp.tile([P, N], F32, tag="bzr")
                for row_t, coord in ((bxr, 0), (byr, 1), (bzr, 2)):
                    xr = sb.tile([1, N], F32, tag="xrow")
                    nc.sync.dma_start(out=xr[:],
                                      in_=xyz_d[c, :, coord:coord + 1]
                                      .rearrange("n c -> c n"))
                    nc.gpsimd.partition_broadcast(out_ap=row_t[:],
                                                  in_ap=xr[:])

                # ---- neighbor search ----
                idxall = bigp.tile([P, NT, NSLOT], I32, tag="idxall")
                nc._dbg_first = [DEBUG_G and c == 0]
                _neighbor_search(nc, tc, sb, aug, bxr, byr, bzr, idxall,
                                 selfc, soffs, obase, sentinel)
                if dbg_d is not None:
                    nc.sync.dma_start(out=dbg_d[c], in_=idxall[:])

                # dma_gather wants idxs int16 wrapped per 16 partitions and
                # replicated across the 8 gpsimd core groups: position
                # i = t*128 + p  ->  idxs[16g + i%16, i//16].
                idx16 = sb.tile([P, NT * NSLOT], mybir.dt.int16, tag="idx16")
                nc.vector.tensor_copy(
                    out=idx16[:],
                    in_=idxall[:].rearrange("p nt s -> p (nt s)"))
                idx_dram = dp.tile([NT * NSLOT, P], mybir.dt.int16,
                                   tag=f"idxdram{c}")
                nc.sync.dma_start(
                    out=idx_dram[:].rearrange("t p -> p t"), in_=idx16[:])
                idxw = bigp.tile([P, ROWS // 16], mybir.dt.int16, tag="idxw")
                flat = idx_dram[:].rearrange("t p -> (t p)") \
                                  .rearrange("(c r) -> r c", r=16)
                for g in range(8):
                    nc.sync.dma_start(out=idxw[16 * g:16 * (g + 1), :],
                                      in_=flat)

                # ---- stack 1 tables ----
                w1dram = dp.tile([ROWS, OC], F32, tag=f"w1dram{c}")
                for jt in range(NT):
                    lhs = augT[:, jt * P:(jt + 1) * P]
                    wsb = sb.tile([P, 8 * OC], F32, tag="wsb")
                    for h in range(2):
                        mm = psmm.tile([P, 512], F32, tag="mm")
                        nc.tensor.matmul(out=mm[:], lhsT=lhs,
                                         rhs=m1t[:, h * 512:(h + 1) * 512],
                                         start=True, stop=True)
                        if h == 0:
                            nc.scalar.copy(out=wsb[:, h * 512:(h + 1) * 512],
                                           in_=mm[:])
                        else:
                            nc.vector.tensor_copy(
                                out=wsb[:, h * 512:(h + 1) * 512], in_=mm[:])
                    nc.sync.dma_start(
                        out=w1dram[:8 * N, :]
                        .rearrange("(s j) o -> j s o", s=8)[jt * P:(jt + 1) * P],
                        in_=wsb[:].rearrange("p (s o) -> p s o", s=8))
                    mm0 = ps.tile([P, P], F32, tag="mm0")
                    nc.tensor.matmul(out=mm0[:], lhsT=augT[0:4,
                                                           jt * P:(jt + 1) * P],
                                     rhs=w01[:], start=True, stop=True)
                    w0sb = sb.tile([P, OC], F32, tag="w0sb")
                    nc.scalar.copy(out=w0sb[:], in_=mm0[:])
                    nc.sync.dma_start(
                        out=w1dram[8 * N + jt * P: 8 * N + (jt + 1) * P, :],
                        in_=w0sb[:])

                # ---- gather 1 + slot sum ----
                g1 = bigp.tile([P, NT, NSLOT, OC], F32, tag="g")
                g1v = g1[:].rearrange("p nt s o -> p (nt s) o")
                for k in range(NSLOT):
                    nc.gpsimd.dma_gather(
                        out_ap=g1v[:, k * 8:(k + 1) * 8, :],
                        in_ap=w1dram[:],
                        idxs_ap=idxw[:, k * 64:(k + 1) * 64],
                        num_idxs=N,
                        num_idxs_reg=N,
                        elem_size=OC,
                        elem_step=OC)
                new1 = bigp.tile([P, NT, OC], F32, tag="new1")
                nc.vector.tensor_reduce(out=new1[:],
                                        in_=g1[:].transpose([0, 1, 3, 2]),
                                        axis=mybir.AxisListType.X,
                                        op=mybir.AluOpType.add)
                if dbgw_d is not None and c == 0:
                    nc.sync.dma_start(out=dbgw_d[:], in_=w1dram[:])
                    nc.sync.dma_start(out=dbgn1_d[:], in_=new1[:])
                    nc.sync.dma_start(
                        out=dbgg1_d[:].rearrange("p nt s o -> p (nt s) o"),
                        in_=g1[:].rearrange("p nt s o -> p (nt s) o"))

                # ---- new1T via PE transposes ----
                new1T = bigp.tile([P, N], F32, tag="new1T")
                for nt in range(NT):
                    tp2 = ps.tile([P, P], F32, tag="tp")
                    nc.tensor.transpose(out=tp2[:], in_=new1[:, nt, :],
                                        identity=ident[:])
                    nc.scalar.copy(out=new1T[:, nt * P:(nt + 1) * P],
                                   in_=tp2[:])

                # ---- stack 2 tables ----
                w2dram = dp.tile([ROWS, OC], F32, tag=f"w2dram{c}")
                for jt in range(NT):
                    wsb2 = sb.tile([P, 8 * OC], F32, tag="wsb")
                    for h in range(2):
                        mm = psmm.tile([P, 512], F32, tag="mm")
                        nc.tensor.matmul(out=mm[:],
                                         lhsT=new1T[:, jt * P:(jt + 1) * P],
                                         rhs=b2t[:, h * 512:(h + 1) * 512],
                                         start=True, stop=False)
                        nc.tensor.matmul(out=mm[:],
                                         lhsT=augT[0:4, jt * P:(jt + 1) * P],
                                         rhs=a2t[:, h * 512:(h + 1) * 512],
                                         start=False, stop=True)
                        if h == 0:
                            nc.scalar.copy(out=wsb2[:, h * 512:(h + 1) * 512],
                                           in_=mm[:])
                        else:
                            nc.vector.tensor_copy(
                                out=wsb2[:, h * 512:(h + 1) * 512], in_=mm[:])
                    nc.sync.dma_start(
                        out=w2dram[:8 * N, :]
                        .rearrange("(s j) o -> j s o", s=8)[jt * P:(jt + 1) * P],
                        in_=wsb2[:].rearrange("p (s o) -> p s o", s=8))
                    mm0 = ps.tile([P, P], F32, tag="mm0")
                    nc.tensor.matmul(out=mm0[:],
                                     lhsT=augT[0:4, jt * P:(jt + 1) * P],
                                     rhs=w02[:], start=True, stop=True)
                    w0sb2 = sb.tile([P, OC], F32, tag="w0sb")
                    nc.scalar.copy(out=w0sb2[:], in_=mm0[:])
                    nc.sync.dma_start(
                        out=w2dram[8 * N + jt * P: 8 * N + (jt + 1) * P, :],
                        in_=w0sb2[:])

                # ---- gather 2 + slot sum + relu + store ----
                g2 = bigp.tile([P, NT, NSLOT, OC], F32, tag="g")
                g2v = g2[:].rearrange("p nt s o -> p (nt s) o")
                for k in range(NSLOT):
                    nc.gpsimd.dma_gather(
                        out_ap=g2v[:, k * 8:(k + 1) * 8, :],
                        in_ap=w2dram[:],
                        idxs_ap=idxw[:, k * 64:(k + 1) * 64],
                        num_idxs=N,
                        num_idxs_reg=N,
                        elem_size=OC,
                        elem_step=OC)
                if dbgw_d is not None and c == 0:
                    nc.sync.dma_start(out=dbgw2_d[:], in_=w2dram[:])
                    nc.sync.dma_start(
                        out=dbgg2_d[:].rearrange("p nt s o -> p (nt s) o"),
                        in_=g2[:].rearrange("p nt s o -> p (nt s) o"))
                    nc.sync.dma_start(out=dbgn1t_d[:], in_=new1T[:])
                new2 = bigp.tile([P, NT, OC], F32, tag="new2")
                nc.vector.tensor_reduce(out=new2[:],
                                        in_=g2[:].transpose([0, 1, 3, 2]),
                                        axis=mybir.AxisListType.X,
                                        op=mybir.AluOpType.add)
                nc.scalar.activation(out=new2[:], in_=new2[:], func=AF.Relu)
                nc.sync.dma_start(
                    out=out_d[c, :, 0:OC].rearrange("(nt p) o -> p nt o", p=P),
                    in_=new2[:])
                ptsr = sb.tile([P, NT, EC], F32, tag="ptsr")
                nc.scalar.activation(out=ptsr[:], in_=aug[:, :, 4:68],
                                     func=AF.Relu)
                nc.sync.dma_start(
                    out=out_d[c, :, OC:].rearrange("(nt p) o -> p nt o", p=P),
                    in_=ptsr[:])


_COMPILED = None


def _get_compiled():
    global _COMPILED
    if _COMPILED is None:
        nc = bacc.Bacc("TRN2", target_bir_lowering=False, debug=False)
        _trace_core_program(nc)
        nc.compile()
        _COMPILED = nc
    return _COMPILED


def _fold_weights(wa, ba, wb, bb, wc, bc):
    WA = [np.asarray(wa[:, :, k], np.float64) for k in (0, 1)]
    WB = [np.asarray(wb[:, :, k], np.float64) for k in (0, 1)]
    WC = [np.asarray(wc[:, :, k], np.float64) for k in (0, 1)]
    Ms = [WC[s // 4] @ WB[(s // 2) % 2] @ WA[s % 2] for s in range(8)]
    c = (np.asarray(bc, np.float64)
         + (WC[0] + WC[1]) @ np.asarray(bb, np.float64)
         + (WC[0] + WC[1]) @ (WB[0] + WB[1]) @ np.asarray(ba, np.float64))
    return [m.astype(np.float32) for m in Ms], c.astype(np.float32)


def _prep_weight_tensors(inp):
    M1, c1 = _fold_weights(inp['w1a'], inp['b1a'], inp['w1b'], inp['b1b'],
                           inp['w1c'], inp['b1c'])
    M2, c2 = _fold_weights(inp['w2a'], inp['b2a'], inp['w2b'], inp['b2b'],
                           inp['w2c'], inp['b2c'])
    m1t = np.zeros((68, 8 * OC), np.float32)
    a2t = np.zeros((4, 8 * OC), np.float32)
    b2t = np.zeros((OC, 8 * OC), np.float32)
    for s in range(8):
        m1t[1:4, s * OC:(s + 1) * OC] = M1[s][:, 0:3].T
        m1t[4:68, s * OC:(s + 1) * OC] = M1[s][:, 3:].T
        a2t[1:4, s * OC:(s + 1) * OC] = M2[s][:, 0:3].T
        b2t[:, s * OC:(s + 1) * OC] = M2[s][:, 3:].T
    A1sum = np.sum([M1[s][:, 0:3] for s in range(8)], axis=0)
    A2sum = np.sum([M2[s][:, 0:3] for s in range(8)], axis=0)
    w01 = np.concatenate([c1[None, :], -A1sum.T], 0).astype(np.float32)
    w02 = np.concatenate([c2[None, :], -A2sum.T], 0).astype(np.float32)
    return dict(m1t=m1t, w01=w01, b2t=b2t, a2t=a2t, w02=w02)


def kernel(xyz, points, w1a, b1a, w1b, b1b, w1c, b1c,
           w2a, b2a, w2b, b2b, w2c, b2c):
    xyz = np.asarray(xyz, np.float32)
    points = np.asarray(points, np.float32)
    B, T, n, _ = xyz.shape
    BT = B * T
    xyz_f = np.ascontiguousarray(xyz.reshape(BT, n, 3))
    pts_f = np.ascontiguousarray(points.reshape(BT, n, EC))
    wdict = _prep_weight_tensors(dict(
        w1a=w1a, b1a=b1a, w1b=w1b, b1b=b1b, w1c=w1c, b1c=b1c,
        w2a=w2a, b2a=b2a, w2b=w2b, b2b=b2b, w2c=w2c, b2c=b2c))

    nc = _get_compiled()
    in_maps = []
    for core in range(NCORES):
        sl = slice(core * CLOUDS, (core + 1) * CLOUDS)
        m = {'xyz': np.ascontiguousarray(xyz_f[sl]),
             'pts': np.ascontiguousarray(pts_f[sl])}
        m.update(wdict)
        in_maps.append(m)
    res = run_bass_kernel_spmd(nc, in_maps, list(range(NCORES)))
    outs = [res.results[i]['out'] for i in range(NCORES)]
    out = np.concatenate(outs, 0).reshape(B, T, n, OC + EC)
    return xyz, out


# revision 26
# speedup vs baseline: 1.0747x; 1.0747x over previous
"""PointSIFT res module on 8 trn2 cores.

Strategy
--------
Data-parallel over BT=16 point clouds: 2 clouds per core.

The three [1,2]-stride-2 convs in each stack have no nonlinearity between
them, so each stack collapses to y[n] = sum_s M_s @ feat[idx[n,s]] + c with
M_s = Wc_{s//4} @ Wb_{(s//2)%2} @ Wa_{s%2} (folded on host in float64).

Per cloud on device:
  1. O(N^2) octant neighbor search, bit-exact vs the jax reference:
     dist = (dx^2+dy^2)+dz^2 in fp32; octant bit b = (fp32(d+1) >= 1);
     invalid (dist<=1e-10 or dist>=jd) pushed to a clamped key region.
     Key G = oct<<28 + (bits(dist)-bits(1e-10)) is order-preserving in dist
     within each octant; per-octant argmin = uint32 wrap-subtract + min;
     indices recovered with one max_index pass; empty octants fall back to
     self (reference semantics).
  2. Transform-then-gather: w_s[j] = M_s @ feat[j] for all j via PE matmuls
     (tables in DRAM, row-major [9216, 128]: 8 slots + 1 self-correction
     slab c - Msum_xyz @ xyz[j]), one indirect-DMA row gather per stack,
     sum over 9 slots, then stack 2 likewise, ReLU, concat with relu(pts).
"""

import numpy as np

import concourse.bass as bass
import concourse.bacc as bacc
import concourse.mybir as mybir
import concourse.tile as tile
from concourse.bass_utils import run_bass_kernel_spmd
from concourse.masks import make_identity

P = 128
NT = 8
N = P * NT            # 1024 points per cloud
CLOUDS = 2            # clouds per core
NCORES = 8
EC = 64
OC = 128
NSLOT = 9             # 8 octant slots + 1 self-correction slab
ROWS = NSLOT * N      # rows in each w-table

F32 = mybir.dt.float32
I32 = mybir.dt.int32
U32 = mybir.dt.uint32

JD = float(np.float32(0.2 * 0.2))          # fp32 value of python 0.2*0.2
# fp32-exact (HW routes ALU immediates through fp32): bits(1e-10)+1.
# valid dist > 1e-10 => bits >= BASE_SUB, so db >= 0 always.
BASE_SUB = 786818816
CLAMP = 0.0625
FOUND_THR = 240000000                       # fp32-exact, between valid/invalid
AF = mybir.ActivationFunctionType
DEBUG_IDX = False
DEBUG_G = False
DEBUG_W = False


def _neighbor_search(nc, tc, sb, aug, bxr, byr, bzr, idxall, selfc, soffs,
                     obase, sentinel):
    """Per-cloud octant argmin. Writes idxall [128, NT, NSLOT] int32 with
    global row ids (slot*1024 + j)."""
    dbg_first = getattr(nc, "_dbg_first", [False])
    for nt in range(NT):
        xn = aug[:, nt, 1:2]
        yn = aug[:, nt, 2:3]
        zn = aug[:, nt, 3:4]

        dx = sb.tile([P, N], F32, tag="dx")
        dy = sb.tile([P, N], F32, tag="dy")
        dz = sb.tile([P, N], F32, tag="dz")
        nc.vector.tensor_scalar(out=dx[:], in0=bxr[:], scalar1=xn,
                                scalar2=None, op0=mybir.AluOpType.subtract)
        nc.vector.tensor_scalar(out=dy[:], in0=byr[:], scalar1=yn,
                                scalar2=None, op0=mybir.AluOpType.subtract)
        nc.vector.tensor_scalar(out=dz[:], in0=bzr[:], scalar1=zn,
                                scalar2=None, op0=mybir.AluOpType.subtract)

        sqx = sb.tile([P, N], F32, tag="sqx")
        sqy = sb.tile([P, N], F32, tag="sqy")
        sqz = sb.tile([P, N], F32, tag="sqz")
        # exact IEEE squares + dist accumulation on gpsimd (fp32 ALU,
        # exact; frees DVE cycles)
        nc.gpsimd.tensor_tensor(out=sqx[:], in0=dx[:], in1=dx[:],
                                op=mybir.AluOpType.mult)
        nc.gpsimd.tensor_tensor(out=sqy[:], in0=dy[:], in1=dy[:],
                                op=mybir.AluOpType.mult)
        nc.gpsimd.tensor_tensor(out=sqz[:], in0=dz[:], in1=dz[:],
                                op=mybir.AluOpType.mult)
        dist = sqx
        nc.gpsimd.tensor_tensor(out=dist[:], in0=dist[:], in1=sqy[:],
                                op=mybir.AluOpType.add)
        nc.gpsimd.tensor_tensor(out=dist[:], in0=dist[:], in1=sqz[:],
                                op=mybir.AluOpType.add)

        # invalid markers: dist <= 1e-10 (self/dup) or dist >= jd.
        # All DVE ALU arithmetic is fp32-domain, so the whole search runs on
        # genuine fp32 values: members of octant o get score dm + 0.0
        # (exact); everything else gets dm + 1 (>= 1 > any valid dist).
        i1 = sqy
        i2 = sqz
        nc.vector.tensor_scalar(out=i1[:], in0=dist[:], scalar1=1e-10,
                                scalar2=64.0, op0=mybir.AluOpType.is_le,
                                op1=mybir.AluOpType.mult)
        nc.vector.tensor_scalar(out=i2[:], in0=dist[:], scalar1=JD,
                                scalar2=64.0, op0=mybir.AluOpType.is_ge,
                                op1=mybir.AluOpType.mult)
        nc.gpsimd.tensor_tensor(out=dist[:], in0=dist[:], in1=i1[:],
                                op=mybir.AluOpType.add)
        nc.gpsimd.tensor_tensor(out=dist[:], in0=dist[:], in1=i2[:],
                                op=mybir.AluOpType.add)
        dm = dist

        # octant bits: b = (fp32(d + 1.0) >= 1.0)  (matches (d+1).int() trunc)
        bx, by, bz = dx, dy, dz
        nc.vector.tensor_scalar(out=bx[:], in0=dx[:], scalar1=1.0,
                                scalar2=1.0, op0=mybir.AluOpType.add,
                                op1=mybir.AluOpType.is_ge)
        nc.vector.tensor_scalar(out=by[:], in0=dy[:], scalar1=1.0,
                                scalar2=1.0, op0=mybir.AluOpType.add,
                                op1=mybir.AluOpType.is_ge)
        nc.vector.tensor_scalar(out=bz[:], in0=dz[:], scalar1=1.0,
                                scalar2=1.0, op0=mybir.AluOpType.add,
                                op1=mybir.AluOpType.is_ge)
        # oct = (2*bx + by)*2 + bz  (fp32 values 0..7)
        nc.vector.scalar_tensor_tensor(out=by[:], in0=bx[:], scalar=2.0,
                                       in1=by[:], op0=mybir.AluOpType.mult,
                                       op1=mybir.AluOpType.add)
        nc.vector.scalar_tensor_tensor(out=bz[:], in0=by[:], scalar=2.0,
                                       in1=bz[:], op0=mybir.AluOpType.mult,
                                       op1=mybir.AluOpType.add)
        octf = bz

        # per-octant: score = dm + [oct != o]; min; first-index via max_index
        m8 = sb.tile([P, 8], F32, tag="m8")
        idx8 = sb.tile([P, 8], U32, tag="idx8")
        for o in range(8):
            score = sb.tile([P, N], F32, tag="score")
            nc.vector.scalar_tensor_tensor(out=score[:], in0=octf[:],
                                           scalar=float(o), in1=dm[:],
                                           op0=mybir.AluOpType.not_equal,
                                           op1=mybir.AluOpType.add)
            nc.vector.tensor_reduce(out=m8[:, o:o + 1], in_=score[:],
                                    axis=mybir.AxisListType.X,
                                    op=mybir.AluOpType.min)
            mi8 = sb.tile([P, 8], U32, tag="mi8")
            nc.vector.max_index(out=mi8[:],
                                in_max=m8[:, o:o + 1].to_broadcast([P, 8]),
                                in_values=score[:])
            nc.vector.tensor_copy(out=idx8[:, o:o + 1], in_=mi8[:, 0:1])

        found = sb.tile([P, 8], I32, tag="found")
        nc.vector.tensor_scalar(out=found[:], in0=m8[:], scalar1=0.5,
                                scalar2=None, op0=mybir.AluOpType.is_lt)
        # self fallback + slot offsets; write into idxall[:, nt, :]
        dst = idxall[:, nt, 0:8]
        nc.vector.tensor_tensor(out=dst, in0=selfc[:, nt:nt + 1]
                                .to_broadcast([P, 8]).bitcast(I32),
                                in1=soffs[:, 0:8], op=mybir.AluOpType.add)
        idx8i = sb.tile([P, 8], I32, tag="idx8i")
        nc.vector.tensor_tensor(out=idx8i[:], in0=idx8[:].bitcast(I32),
                                in1=soffs[:, 0:8], op=mybir.AluOpType.add)
        nc.vector.copy_predicated(out=dst, mask=found[:], data=idx8i[:])
        nc.vector.tensor_scalar(out=idxall[:, nt, 8:9],
                                in0=selfc[:, nt:nt + 1], scalar1=8 * N,
                                scalar2=None, op0=mybir.AluOpType.add)


def _trace_core_program(nc):
    """Trace the full per-core tile program (2 clouds)."""
    xyz_d = nc.dram_tensor('xyz', [CLOUDS, N, 3], F32, kind="ExternalInput")
    pts_d = nc.dram_tensor('pts', [CLOUDS, N, EC], F32, kind="ExternalInput")
    m1t_d = nc.dram_tensor('m1t', [68, 8 * OC], F32, kind="ExternalInput")
    w01_d = nc.dram_tensor('w01', [4, OC], F32, kind="ExternalInput")
    b2t_d = nc.dram_tensor('b2t', [OC, 8 * OC], F32, kind="ExternalInput")
    a2t_d = nc.dram_tensor('a2t', [4, 8 * OC], F32, kind="ExternalInput")
    w02_d = nc.dram_tensor('w02', [4, OC], F32, kind="ExternalInput")
    out_d = nc.dram_tensor('out', [CLOUDS, N, OC + EC], F32,
                           kind="ExternalOutput")
    dbg_d = None
    if DEBUG_IDX:
        dbg_d = nc.dram_tensor('dbg_idx', [CLOUDS, P, NT, NSLOT], I32,
                               kind="ExternalOutput")
    dbgw_d = None
    if DEBUG_W:
        dbgw_d = nc.dram_tensor('dbg_w1', [ROWS, OC], F32,
                                kind="ExternalOutput")
        dbgn1_d = nc.dram_tensor('dbg_new1', [P, NT, OC], F32,
                                 kind="ExternalOutput")
        dbgg1_d = nc.dram_tensor('dbg_g1', [P, NT, NSLOT, OC], F32,
                                 kind="ExternalOutput")
        dbgw2_d = nc.dram_tensor('dbg_w2', [ROWS, OC], F32,
                                 kind="ExternalOutput")
        dbgg2_d = nc.dram_tensor('dbg_g2', [P, NT, NSLOT, OC], F32,
                                 kind="ExternalOutput")
        dbgn1t_d = nc.dram_tensor('dbg_n1t', [P, N], F32,
                                  kind="ExternalOutput")
    dbgg_d = None
    if DEBUG_G:
        dbgg_d = nc.dram_tensor('dbg_g', [P, N], I32, kind="ExternalOutput")
        dbgm_d = nc.dram_tensor('dbg_mins', [P, 8], I32,
                                kind="ExternalOutput")
        dbgt_d = nc.dram_tensor('dbg_tgt', [P, 8], I32, kind="ExternalOutput")
        dbgi_d = nc.dram_tensor('dbg_i8', [P, 8], I32, kind="ExternalOutput")
        nc._dbg_tensors = (dbgg_d, dbgm_d, dbgt_d, dbgi_d)
    nc._dbg_g = dbgg_d

    with tile.TileContext(nc) as tc:
        with tc.tile_pool(name="const", bufs=1) as cp, \
             tc.tile_pool(name="sb", bufs=2) as sb, \
             tc.tile_pool(name="big", bufs=1) as bigp, \
             tc.tile_pool(name="ps", bufs=2, space="PSUM") as ps, \
             tc.tile_pool(name="psmm", bufs=4, space="PSUM") as psmm, \
             tc.tile_pool(name="dram", bufs=1, space="DRAM") as dp:

            ident = cp.tile([P, P], F32)
            make_identity(nc, ident[:])
            m1t = cp.tile([68, 8 * OC], F32)
            nc.sync.dma_start(out=m1t[:], in_=m1t_d[:])
            w01 = cp.tile([4, OC], F32)
            nc.sync.dma_start(out=w01[:], in_=w01_d[:])
            b2t = cp.tile([OC, 8 * OC], F32)
            nc.sync.dma_start(out=b2t[:], in_=b2t_d[:])
            a2t = cp.tile([4, 8 * OC], F32)
            nc.sync.dma_start(out=a2t[:], in_=a2t_d[:])
            w02 = cp.tile([4, OC], F32)
            nc.sync.dma_start(out=w02[:], in_=w02_d[:])

            obase = cp.tile([P, 8], I32)
            nc.gpsimd.iota(out=obase[:], pattern=[[1, 8]], base=0,
                           channel_multiplier=0)
            nc.vector.tensor_scalar(out=obase[:], in0=obase[:], scalar1=28,
                                    scalar2=None,
                                    op0=mybir.AluOpType.logical_shift_left)
            soffs = cp.tile([P, 8], I32)
            nc.gpsimd.iota(out=soffs[:], pattern=[[N, 8]], base=0,
                           channel_multiplier=0)
            selfc = cp.tile([P, NT], I32)
            nc.gpsimd.iota(out=selfc[:], pattern=[[P, NT]], base=0,
                           channel_multiplier=1)
            sentinel = cp.tile([P, 8], U32)
            nc.gpsimd.memset(sentinel[:], 0xFFFFFFFF)

            for c in range(CLOUDS):
                # ---- load + build aug [p, nt, 68] = [1, xyz, pts] ----
                aug = sb.tile([P, NT, 68], F32, tag="aug")
                nc.gpsimd.memset(aug[:, :, 0:1], 1.0)
                nc.sync.dma_start(
                    out=aug[:, :, 1:4],
                    in_=xyz_d[c].rearrange("(nt p) c -> p nt c", p=P))
                nc.sync.dma_start(
                    out=aug[:, :, 4:68],
                    in_=pts_d[c].rearrange("(nt p) c -> p nt c", p=P))

                # ---- augT [68, 1024] via PE transposes ----
                augT = bigp.tile([68, N], F32, tag="augT")
                for nt in range(NT):
                    tp = ps.tile([P, P], F32, tag="tp")
                    nc.tensor.transpose(out=tp[:68, :], in_=aug[:, nt, :],
                                        identity=ident[:])
                    nc.scalar.copy(out=augT[:, nt * P:(nt + 1) * P],
                                   in_=tp[:68, :])

                # ---- broadcast coordinate rows ----
                bxr = bigp.tile([P, N], F32, tag="bxr")
                byr = bigp.tile([P, N], F32, tag="byr")
                bzr = bigp.tile([P, N], F32, tag="bzr")
                for row_t, coord in ((bxr, 0), (byr, 1), (bzr, 2)):
                    xr = sb.tile([1, N], F32, tag="xrow")
                    nc.sync.dma_start(out=xr[:],
                                      in_=xyz_d[c, :, coord:coord + 1]
                                      .rearrange("n c -> c n"))
                    nc.gpsimd.partition_broadcast(out_ap=row_t[:],
                                                  in_ap=xr[:])

                # ---- neighbor search ----
                idxall = bigp.tile([P, NT, NSLOT], I32, tag="idxall")
                nc._dbg_first = [DEBUG_G and c == 0]
                _neighbor_search(nc, tc, sb, aug, bxr, byr, bzr, idxall,
                                 selfc, soffs, obase, sentinel)
                if dbg_d is not None:
                    nc.sync.dma_start(out=dbg_d[c], in_=idxall[:])

                # dma_gather wants idxs int16 wrapped per 16 partitions and
                # replicated across the 8 gpsimd core groups: position
                # i = t*128 + p  ->  idxs[16g + i%16, i//16].
                idx16 = sb.tile([P, NT * NSLOT], mybir.dt.int16, tag="idx16")
                nc.vector.tensor_copy(
                    out=idx16[:],
                    in_=idxall[:].rearrange("p nt s -> p (nt s)"))
                idx_dram = dp.tile([NT * NSLOT, P], mybir.dt.int16,
                                   tag=f"idxdram{c}")
                nc.sync.dma_start(
                    out=idx_dram[:].rearrange("t p -> p t"), in_=idx16[:])
                idxw = bigp.tile([P, ROWS // 16], mybir.dt.int16, tag="idxw")
                flat = idx_dram[:].rearrange("t p -> (t p)") \
                                  .rearrange("(c r) -> r c", r=16)
                for g in range(8):
                    nc.sync.dma_start(out=idxw[16 * g:16 * (g + 1), :],
                                      in_=flat)

                # ---- stack 1 tables ----
                w1dram = dp.tile([ROWS, OC], F32, tag=f"w1dram{c}")
                for jt in range(NT):
                    lhs = augT[:, jt * P:(jt + 1) * P]
                    wsb = sb.tile([P, 8 * OC], F32, tag="wsb")
                    for h in range(2):
                        mm = psmm.tile([P, 512], F32, tag="mm")
                        nc.tensor.matmul(out=mm[:], lhsT=lhs,
                                         rhs=m1t[:, h * 512:(h + 1) * 512],
                                         start=True, stop=True)
                        if h == 0:
                            nc.scalar.copy(out=wsb[:, h * 512:(h + 1) * 512],
                                           in_=mm[:])
                        else:
                            nc.vector.tensor_copy(
                                out=wsb[:, h * 512:(h + 1) * 512], in_=mm[:])
                    nc.sync.dma_start(
                        out=w1dram[:8 * N, :]
                        .rearrange("(s j) o -> j s o", s=8)[jt * P:(jt + 1) * P],
                        in_=wsb[:].rearrange("p (s o) -> p s o", s=8))
                    mm0 = ps.tile([P, P], F32, tag="mm0")
                    nc.tensor.matmul(out=mm0[:], lhsT=augT[0:4,
                                                           jt * P:(jt + 1) * P],
                                     rhs=w01[:], start=True, stop=True)
                    w0sb = sb.tile([P, OC], F32, tag="w0sb")
                    nc.scalar.copy(out=w0sb[:], in_=mm0[:])
                    nc.sync.dma_start(
                        out=w1dram[8 * N + jt * P: 8 * N + (jt + 1) * P, :],
                        in_=w0sb[:])

                # ---- gather 1 + slot sum ----
                g1 = bigp.tile([P, NT, NSLOT, OC], F32, tag="g")
                g1v = g1[:].rearrange("p nt s o -> p (nt s) o")
                for k in range(NSLOT):
                    nc.gpsimd.dma_gather(
                        out_ap=g1v[:, k * 8:(k + 1) * 8, :],
                        in_ap=w1dram[:],
                        idxs_ap=idxw[:, k * 64:(k + 1) * 64],
                        num_idxs=N,
                        num_idxs_reg=N,
                        elem_size=OC,
                        elem_step=OC)
                new1 = bigp.tile([P, NT, OC], F32, tag="new1")
                nc.vector.tensor_reduce(out=new1[:],
                                        in_=g1[:].transpose([0, 1, 3, 2]),
                                        axis=mybir.AxisListType.X,
                                        op=mybir.AluOpType.add)
                if dbgw_d is not None and c == 0:
                    nc.sync.dma_start(out=dbgw_d[:], in_=w1dram[:])
                    nc.sync.dma_start(out=dbgn1_d[:], in_=new1[:])
                    nc.sync.dma_start(
                        out=dbgg1_d[:].rearrange("p nt s o -> p (nt s) o"),
                        in_=g1[:].rearrange("p nt s o -> p (nt s) o"))

                # ---- new1T via PE transposes ----
                new1T = bigp.tile([P, N], F32, tag="new1T")
                for nt in range(NT):
                    tp2 = ps.tile([P, P], F32, tag="tp")
                    nc.tensor.transpose(out=tp2[:], in_=new1[:, nt, :],
                                        identity=ident[:])
                    nc.scalar.copy(out=new1T[:, nt * P:(nt + 1) * P],
                                   in_=tp2[:])

                # ---- stack 2 tables ----
                w2dram = dp.tile([ROWS, OC], F32, tag=f"w2dram{c}")
                for jt in range(NT):
                    wsb2 = sb.tile([P, 8 * OC], F32, tag="wsb")
                    for h in range(2):
                        mm = psmm.tile([P, 512], F32, tag="mm")
                        nc.tensor.matmul(out=mm[:],
                                         lhsT=new1T[:, jt * P:(jt + 1) * P],
                                         rhs=b2t[:, h * 512:(h + 1) * 512],
                                         start=True, stop=False)
                        nc.tensor.matmul(out=mm[:],
                                         lhsT=augT[0:4, jt * P:(jt + 1) * P],
                                         rhs=a2t[:, h * 512:(h + 1) * 512],
                                         start=False, stop=True)
                        if h == 0:
                            nc.scalar.copy(out=wsb2[:, h * 512:(h + 1) * 512],
                                           in_=mm[:])
                        else:
                            nc.vector.tensor_copy(
                                out=wsb2[:, h * 512:(h + 1) * 512], in_=mm[:])
                    nc.sync.dma_start(
                        out=w2dram[:8 * N, :]
                        .rearrange("(s j) o -> j s o", s=8)[jt * P:(jt + 1) * P],
                        in_=wsb2[:].rearrange("p (s o) -> p s o", s=8))
                    mm0 = ps.tile([P, P], F32, tag="mm0")
                    nc.tensor.matmul(out=mm0[:],
                                     lhsT=augT[0:4, jt * P:(jt + 1) * P],
                                     rhs=w02[:], start=True, stop=True)
                    w0sb2 = sb.tile([P, OC], F32, tag="w0sb")
                    nc.scalar.copy(out=w0sb2[:], in_=mm0[:])
                    nc.sync.dma_start(
                        out=w2dram[8 * N + jt * P: 8 * N + (jt + 1) * P, :],
                        in_=w0sb2[:])

                # ---- gather 2 + slot sum + relu + store ----
                g2 = bigp.tile([P, NT, NSLOT, OC], F32, tag="g")
                g2v = g2[:].rearrange("p nt s o -> p (nt s) o")
                for k in range(NSLOT):
                    nc.gpsimd.dma_gather(
                        out_ap=g2v[:, k * 8:(k + 1) * 8, :],
                        in_ap=w2dram[:],
                        idxs_ap=idxw[:, k * 64:(k + 1) * 64],
                        num_idxs=N,
                        num_idxs_reg=N,
                        elem_size=OC,
                        elem_step=OC)
                if dbgw_d is not None and c == 0:
                    nc.sync.dma_start(out=dbgw2_d[:], in_=w2dram[:])
                    nc.sync.dma_start(
                        out=dbgg2_d[:].rearrange("p nt s o -> p (nt s) o"),
                        in_=g2[:].rearrange("p nt s o -> p (nt s) o"))
                    nc.sync.dma_start(out=dbgn1t_d[:], in_=new1T[:])
                new2 = bigp.tile([P, NT, OC], F32, tag="new2")
                nc.vector.tensor_reduce(out=new2[:],
                                        in_=g2[:].transpose([0, 1, 3, 2]),
                                        axis=mybir.AxisListType.X,
                                        op=mybir.AluOpType.add)
                nc.scalar.activation(out=new2[:], in_=new2[:], func=AF.Relu)
                nc.sync.dma_start(
                    out=out_d[c, :, 0:OC].rearrange("(nt p) o -> p nt o", p=P),
                    in_=new2[:])
                ptsr = sb.tile([P, NT, EC], F32, tag="ptsr")
                nc.scalar.activation(out=ptsr[:], in_=aug[:, :, 4:68],
                                     func=AF.Relu)
                nc.sync.dma_start(
                    out=out_d[c, :, OC:].rearrange("(nt p) o -> p nt o", p=P),
                    in_=ptsr[:])


_COMPILED = None


def _get_compiled():
    global _COMPILED
    if _COMPILED is None:
        nc = bacc.Bacc("TRN2", target_bir_lowering=False, debug=False)
        _trace_core_program(nc)
        nc.compile()
        _COMPILED = nc
    return _COMPILED


def _fold_weights(wa, ba, wb, bb, wc, bc):
    WA = [np.asarray(wa[:, :, k], np.float64) for k in (0, 1)]
    WB = [np.asarray(wb[:, :, k], np.float64) for k in (0, 1)]
    WC = [np.asarray(wc[:, :, k], np.float64) for k in (0, 1)]
    Ms = [WC[s // 4] @ WB[(s // 2) % 2] @ WA[s % 2] for s in range(8)]
    c = (np.asarray(bc, np.float64)
         + (WC[0] + WC[1]) @ np.asarray(bb, np.float64)
         + (WC[0] + WC[1]) @ (WB[0] + WB[1]) @ np.asarray(ba, np.float64))
    return [m.astype(np.float32) for m in Ms], c.astype(np.float32)


def _prep_weight_tensors(inp):
    M1, c1 = _fold_weights(inp['w1a'], inp['b1a'], inp['w1b'], inp['b1b'],
                           inp['w1c'], inp['b1c'])
    M2, c2 = _fold_weights(inp['w2a'], inp['b2a'], inp['w2b'], inp['b2b'],
                           inp['w2c'], inp['b2c'])
    m1t = np.zeros((68, 8 * OC), np.float32)
    a2t = np.zeros((4, 8 * OC), np.float32)
    b2t = np.zeros((OC, 8 * OC), np.float32)
    for s in range(8):
        m1t[1:4, s * OC:(s + 1) * OC] = M1[s][:, 0:3].T
        m1t[4:68, s * OC:(s + 1) * OC] = M1[s][:, 3:].T
        a2t[1:4, s * OC:(s + 1) * OC] = M2[s][:, 0:3].T
        b2t[:, s * OC:(s + 1) * OC] = M2[s][:, 3:].T
    A1sum = np.sum([M1[s][:, 0:3] for s in range(8)], axis=0)
    A2sum = np.sum([M2[s][:, 0:3] for s in range(8)], axis=0)
    w01 = np.concatenate([c1[None, :], -A1sum.T], 0).astype(np.float32)
    w02 = np.concatenate([c2[None, :], -A2sum.T], 0).astype(np.float32)
    return dict(m1t=m1t, w01=w01, b2t=b2t, a2t=a2t, w02=w02)


def kernel(xyz, points, w1a, b1a, w1b, b1b, w1c, b1c,
           w2a, b2a, w2b, b2b, w2c, b2c):
    xyz = np.asarray(xyz, np.float32)
    points = np.asarray(points, np.float32)
    B, T, n, _ = xyz.shape
    BT = B * T
    xyz_f = np.ascontiguousarray(xyz.reshape(BT, n, 3))
    pts_f = np.ascontiguousarray(points.reshape(BT, n, EC))
    wdict = _prep_weight_tensors(dict(
        w1a=w1a, b1a=b1a, w1b=w1b, b1b=b1b, w1c=w1c, b1c=b1c,
        w2a=w2a, b2a=b2a, w2b=w2b, b2b=b2b, w2c=w2c, b2c=b2c))

    nc = _get_compiled()
    in_maps = []
    for core in range(NCORES):
        sl = slice(core * CLOUDS, (core + 1) * CLOUDS)
        m = {'xyz': np.ascontiguousarray(xyz_f[sl]),
             'pts': np.ascontiguousarray(pts_f[sl])}
        m.update(wdict)
        in_maps.append(m)
    res = run_bass_kernel_spmd(nc, in_maps, list(range(NCORES)))
    outs = [res.results[i]['out'] for i in range(NCORES)]
    out = np.concatenate(outs, 0).reshape(B, T, n, OC + EC)
    return xyz, out
